# revision 19
# baseline (speedup 1.0000x reference)
"""COLoRA linear kernel for 8 Trainium2 NeuronCores.

Reference computation (per batch element b with task t = task_ids[b]):

    out[b] = x[b] @ W.T + bias
           + cw      * 2 * (x[b] @ shared_A.T)    @ shared_B.T
           + (1-cw)  * 2 * (x[b] @ expert_A[t].T) @ expert_B[t].T
    cw = sigmoid(collab_w)

The rank-8 adapters fold exactly into the dense weight (associativity):

    W_eff[b] = W + cw*2*(shared_B @ shared_A) + (1-cw)*2*(expert_B[t] @ expert_A[t])
    out[b]   = x[b] @ W_eff[b].T + bias

so the device kernel is a single memory-bound GEMM per core. Sharding is
data-parallel over batch: core c handles batch element c (B == n_cores == 8).
The MoE routing (task_ids gather) happens on the host at dispatch time.

x is pre-transposed on the host to [d_in, s] so that the contraction dim
lands on SBUF partitions with fully contiguous DMA access patterns — no
on-chip transpose is needed.  Matmuls run in float32r (fp32 storage,
single-pass reduced-precision PE mode: 1 cycle/row at moving dim >= 256).
"""

import os

import numpy as np

import concourse.bass as bass
import concourse.tile as tile
from concourse import bacc, mybir
from concourse.bass_utils import run_bass_kernel_spmd

try:  # tracing (BASS_TRACE) needs the axon NTFF hook; scrub if unavailable
    from antenv.axon_hooks import get_axon_ntff_profile_hook  # noqa: F401
except ImportError:
    os.environ.pop("BASS_TRACE", None)

N_CORES = 8
S = 4096        # rows per core (sequence length; one batch element per core)
D_IN = 1024
D_OUT = 1024
KC = D_IN // 128   # contraction chunks of 128
S_MACRO = 512      # s rows loaded per x DMA
N_HALF = 512       # psum free dim (one bank)
SCALING = 2.0      # lora alpha/r = 16/8

# bf16 fallback switch (halves input DMA bytes, looser numerics). float32r
# measured accurate and memory-roofline-bound, so keep fp32 storage.
MM_DT = mybir.dt.float32r

_PROGRAM = None
LAST_RESULTS = None  # test harness introspection (exec_time_ns when traced)


def _build_program():
    f32 = mybir.dt.float32
    nc = bacc.Bacc("TRN2", debug=False, num_devices=N_CORES)

    xt_d = nc.dram_tensor("xt", [D_IN, S], MM_DT, kind="ExternalInput").ap()
    wt_d = nc.dram_tensor("wt", [D_IN, D_OUT], MM_DT, kind="ExternalInput").ap()
    bb_d = nc.dram_tensor("bb", [128, D_OUT], f32, kind="ExternalInput").ap()
    out_d = nc.dram_tensor("out", [S, D_OUT], f32, kind="ExternalOutput").ap()

    # contraction dim on partitions, chunked by 128
    xt_v = xt_d.rearrange("(k p) s -> p k s", p=128)      # [128, KC, S]
    wt_v = wt_d.rearrange("(k p) o -> p k o", p=128)      # [128, KC, D_OUT]
    # output rows s = t*S_MACRO + u*128 + p
    out_v = out_d.rearrange(
        "(t u p) o -> t u p o", u=S_MACRO // 128, p=128
    )  # [T, 4, 128, D_OUT]

    with tile.TileContext(nc) as tc:
        with (
            tc.tile_pool(name="const", bufs=1) as cpool,
            tc.tile_pool(name="xin", bufs=3) as xpool,
            tc.tile_pool(name="outp", bufs=4) as opool,
            tc.tile_pool(name="psum", bufs=8, space="PSUM") as ppool,
        ):
            # PE HAM warmup: dummy matmuls with no DMA deps ramp the PE
            # clock (1.2 -> 2.4 GHz takes ~3.4us of sustained activity)
            # while the first input DMAs are still in flight.
            warm_w = cpool.tile([128, 128], f32)
            warm_x = cpool.tile([128, 256], f32)
            nc.gpsimd.memset(warm_w[:], 0.0)
            nc.gpsimd.memset(warm_x[:], 0.0)
            warm_ps = ppool.tile([128, N_HALF], f32, tag="ps")
            for _ in range(12):
                nc.tensor.matmul(
                    warm_ps[:, :256], warm_w[:], warm_x[:], start=True, stop=True
                )

            # weights per k-chunk on the ACT HWDGE ring so chunk 0 is
            # available ~2us after issue instead of after the full 4MiB
            wtile = cpool.tile([128, KC, D_OUT], MM_DT)
            for k in range(KC):
                nc.scalar.dma_start(wtile[:, k, :], wt_v[:, k, :])
            btile = cpool.tile([128, D_OUT], f32)
            nc.scalar.dma_start(btile[:], bb_d[:])

            NU = S_MACRO // 128
            NH = D_OUT // N_HALF
            for t in range(S // S_MACRO):
                xtile = xpool.tile([128, KC, S_MACRO], MM_DT)
                # split loads: matmuls on early k chunks start before the
                # later chunks arrive (finest split on the first tile,
                # which gates the pipeline ramp)
                s_sl = slice(t * S_MACRO, (t + 1) * S_MACRO)
                if t == 0:
                    for k in range(KC):
                        nc.sync.dma_start(xtile[:, k, :], xt_v[:, k, s_sl])
                else:
                    nc.sync.dma_start(
                        xtile[:, : KC // 2, :], xt_v[:, : KC // 2, s_sl]
                    )
                    nc.sync.dma_start(
                        xtile[:, KC // 2 :, :], xt_v[:, KC // 2 :, s_sl]
                    )
                if t == 0:
                    # ramp macro: k outermost with all 8 psum groups open —
                    # each arriving (x[k], W[k]) chunk pair feeds 8 matmuls
                    # (~1.8us PE work per ~1.9us of DMA), so the PE never
                    # idles long enough to re-throttle while the front-load
                    # streams in.
                    otiles, pss = [], []
                    for u in range(NU):
                        otile = opool.tile([128, D_OUT], f32)
                        otiles.append(otile)
                        for _h in range(NH):
                            ps = ppool.tile([128, N_HALF], f32, tag="ps")
                            pss.append(ps)
                    for k in range(KC):
                        for u in range(NU):
                            for h in range(NH):
                                nc.tensor.matmul(
                                    pss[u * NH + h][:],
                                    xtile[:, k, u * 128 : (u + 1) * 128],
                                    wtile[:, k, h * N_HALF : (h + 1) * N_HALF],
                                    start=(k == 0),
                                    stop=(k == KC - 1),
                                )
                    for u in range(NU):
                        for h in range(NH):
                            nc.vector.tensor_add(
                                otiles[u][:, h * N_HALF : (h + 1) * N_HALF],
                                pss[u * NH + h][:],
                                btile[:, h * N_HALF : (h + 1) * N_HALF],
                            )
                        store_eng = nc.scalar if u % 2 == 0 else nc.sync
                        store_eng.dma_start(out_v[t, u], otiles[u][:])
                    continue
                for u in range(NU):
                    otile = opool.tile([128, D_OUT], f32)
                    pss = []
                    for _h in range(NH):
                        ps = ppool.tile([128, N_HALF], f32, tag="ps")
                        pss.append(ps)
                    for k in range(KC):
                        # both output halves per k: consecutive matmuls
                        # share the stationary lhsT, halving LDW pressure
                        for h in range(NH):
                            nc.tensor.matmul(
                                pss[h][:],
                                xtile[:, k, u * 128 : (u + 1) * 128],  # lhsT [K,M]
                                wtile[:, k, h * N_HALF : (h + 1) * N_HALF],  # rhs [K,N]
                                start=(k == 0),
                                stop=(k == KC - 1),
                            )
                    for h in range(NH):
                        # evacuate psum with fused bias add
                        nc.vector.tensor_add(
                            otile[:, h * N_HALF : (h + 1) * N_HALF],
                            pss[h][:],
                            btile[:, h * N_HALF : (h + 1) * N_HALF],
                        )
                    if t == S // S_MACRO - 1:
                        # final macro: store halves on both rings as soon
                        # as each bias-add lands — halves the last flush
                        # the exit drain waits on
                        for h in range(NH):
                            eng = nc.scalar if h == 0 else nc.sync
                            eng.dma_start(
                                out_v[t, u][:, h * N_HALF : (h + 1) * N_HALF],
                                otile[:, h * N_HALF : (h + 1) * N_HALF],
                            )
                    else:
                        # alternate store rings to halve store-issue queuing
                        store_eng = nc.scalar if (t * 4 + u) % 2 == 0 else nc.sync
                        store_eng.dma_start(out_v[t, u], otile[:])

    nc.compile()
    return nc


def _get_program():
    global _PROGRAM
    if _PROGRAM is None:
        _PROGRAM = _build_program()
    return _PROGRAM


def kernel(x, task_ids, W, b, shared_A, shared_B, expert_A, expert_B, collab_w):
    global LAST_RESULTS
    x = np.asarray(x, dtype=np.float32)
    task_ids = np.asarray(task_ids)
    W = np.asarray(W, dtype=np.float32)
    b = np.asarray(b, dtype=np.float32)
    B = x.shape[0]
    assert B == N_CORES and x.shape[1:] == (S, D_IN)

    cw = np.float32(1.0 / (1.0 + np.exp(-np.float64(collab_w))))
    w_shared = (
        W
        + np.float32(cw * SCALING)
        * (np.asarray(shared_B, np.float32) @ np.asarray(shared_A, np.float32))
    ).astype(np.float32)
    ce = np.float32((1.0 - cw) * SCALING)

    np_in = mybir.dt.np(MM_DT)
    bb = np.ascontiguousarray(np.broadcast_to(b, (128, D_OUT)), dtype=np.float32)
    in_maps = []
    for bi in range(B):
        t = int(task_ids[bi])
        w_eff = w_shared + ce * (
            np.asarray(expert_B[t], np.float32) @ np.asarray(expert_A[t], np.float32)
        )
        in_maps.append(
            {
                "xt": np.ascontiguousarray(x[bi].T).astype(np_in),
                "wt": np.ascontiguousarray(w_eff.T).astype(np_in),
                "bb": bb,
            }
        )

    nc = _get_program()
    LAST_RESULTS = run_bass_kernel_spmd(nc, in_maps, list(range(N_CORES)))
    out = np.stack(
        [LAST_RESULTS.results[c]["out"] for c in range(N_CORES)], axis=0
    )
    return np.ascontiguousarray(out, dtype=np.float32)


# revision 20
# speedup vs baseline: 1.0024x; 1.0024x over previous
"""COLoRA linear kernel for 8 Trainium2 NeuronCores.

Reference computation (per batch element b with task t = task_ids[b]):

    out[b] = x[b] @ W.T + bias
           + cw      * 2 * (x[b] @ shared_A.T)    @ shared_B.T
           + (1-cw)  * 2 * (x[b] @ expert_A[t].T) @ expert_B[t].T
    cw = sigmoid(collab_w)

The rank-8 adapters fold exactly into the dense weight (associativity):

    W_eff[b] = W + cw*2*(shared_B @ shared_A) + (1-cw)*2*(expert_B[t] @ expert_A[t])
    out[b]   = x[b] @ W_eff[b].T + bias

so the device kernel is a single memory-bound GEMM per core. Sharding is
data-parallel over batch: core c handles batch element c (B == n_cores == 8).
The MoE routing (task_ids gather) happens on the host at dispatch time.

x is pre-transposed on the host to [d_in, s] so that the contraction dim
lands on SBUF partitions with fully contiguous DMA access patterns — no
on-chip transpose is needed.  Matmuls run in float32r (fp32 storage,
single-pass reduced-precision PE mode: 1 cycle/row at moving dim >= 256).
"""

import os

import numpy as np

import concourse.bass as bass
import concourse.tile as tile
from concourse import bacc, mybir
from concourse.bass_utils import run_bass_kernel_spmd

try:  # tracing (BASS_TRACE) needs the axon NTFF hook; scrub if unavailable
    from antenv.axon_hooks import get_axon_ntff_profile_hook  # noqa: F401
except ImportError:
    os.environ.pop("BASS_TRACE", None)

N_CORES = 8
S = 4096        # rows per core (sequence length; one batch element per core)
D_IN = 1024
D_OUT = 1024
KC = D_IN // 128   # contraction chunks of 128
S_MACRO = 512      # s rows loaded per x DMA
N_HALF = 512       # psum free dim (one bank)
SCALING = 2.0      # lora alpha/r = 16/8

# bf16 fallback switch (halves input DMA bytes, looser numerics). float32r
# measured accurate and memory-roofline-bound, so keep fp32 storage.
MM_DT = mybir.dt.float32r

_PROGRAM = None
LAST_RESULTS = None  # test harness introspection (exec_time_ns when traced)


def _build_program():
    f32 = mybir.dt.float32
    nc = bacc.Bacc("TRN2", debug=False, num_devices=N_CORES)

    xt_d = nc.dram_tensor("xt", [D_IN, S], MM_DT, kind="ExternalInput").ap()
    wt_d = nc.dram_tensor("wt", [D_IN, D_OUT], MM_DT, kind="ExternalInput").ap()
    bb_d = nc.dram_tensor("bb", [128, D_OUT], f32, kind="ExternalInput").ap()
    out_d = nc.dram_tensor("out", [S, D_OUT], f32, kind="ExternalOutput").ap()

    # contraction dim on partitions, chunked by 128
    xt_v = xt_d.rearrange("(k p) s -> p k s", p=128)      # [128, KC, S]
    wt_v = wt_d.rearrange("(k p) o -> p k o", p=128)      # [128, KC, D_OUT]
    # output rows s = t*S_MACRO + u*128 + p
    out_v = out_d.rearrange(
        "(t u p) o -> t u p o", u=S_MACRO // 128, p=128
    )  # [T, 4, 128, D_OUT]

    with tile.TileContext(nc) as tc:
        with (
            tc.tile_pool(name="const", bufs=1) as cpool,
            tc.tile_pool(name="xin", bufs=3) as xpool,
            tc.tile_pool(name="outp", bufs=4) as opool,
            tc.tile_pool(name="psum", bufs=8, space="PSUM") as ppool,
        ):
            # PE HAM warmup: dummy matmuls with no DMA deps ramp the PE
            # clock (1.2 -> 2.4 GHz takes ~3.4us of sustained activity)
            # while the first input DMAs are still in flight.
            warm_w = cpool.tile([128, 128], f32)
            warm_x = cpool.tile([128, 256], f32)
            nc.gpsimd.memset(warm_w[:], 0.0)
            nc.gpsimd.memset(warm_x[:], 0.0)
            warm_ps = ppool.tile([128, N_HALF], f32, tag="ps")
            for _ in range(8):
                nc.tensor.matmul(
                    warm_ps[:, :256], warm_w[:], warm_x[:], start=True, stop=True
                )

            # weights per k-chunk on the ACT HWDGE ring so chunk 0 is
            # available ~2us after issue instead of after the full 4MiB
            wtile = cpool.tile([128, KC, D_OUT], MM_DT)
            for k in range(KC):
                nc.scalar.dma_start(wtile[:, k, :], wt_v[:, k, :])
            btile = cpool.tile([128, D_OUT], f32)
            nc.scalar.dma_start(btile[:], bb_d[:])

            NU = S_MACRO // 128
            NH = D_OUT // N_HALF
            for t in range(S // S_MACRO):
                xtile = xpool.tile([128, KC, S_MACRO], MM_DT)
                # split loads: matmuls on early k chunks start before the
                # later chunks arrive (finest split on the first tile,
                # which gates the pipeline ramp)
                s_sl = slice(t * S_MACRO, (t + 1) * S_MACRO)
                if t == 0:
                    for k in range(KC):
                        nc.sync.dma_start(xtile[:, k, :], xt_v[:, k, s_sl])
                else:
                    nc.sync.dma_start(
                        xtile[:, : KC // 2, :], xt_v[:, : KC // 2, s_sl]
                    )
                    nc.sync.dma_start(
                        xtile[:, KC // 2 :, :], xt_v[:, KC // 2 :, s_sl]
                    )
                if t == 0:
                    # ramp macro: k outermost with all 8 psum groups open —
                    # each arriving (x[k], W[k]) chunk pair feeds 8 matmuls
                    # (~1.8us PE work per ~1.9us of DMA), so the PE never
                    # idles long enough to re-throttle while the front-load
                    # streams in.
                    otiles, pss = [], []
                    for u in range(NU):
                        otile = opool.tile([128, D_OUT], f32)
                        otiles.append(otile)
                        for _h in range(NH):
                            ps = ppool.tile([128, N_HALF], f32, tag="ps")
                            pss.append(ps)
                    for k in range(KC):
                        for u in range(NU):
                            for h in range(NH):
                                nc.tensor.matmul(
                                    pss[u * NH + h][:],
                                    xtile[:, k, u * 128 : (u + 1) * 128],
                                    wtile[:, k, h * N_HALF : (h + 1) * N_HALF],
                                    start=(k == 0),
                                    stop=(k == KC - 1),
                                )
                    for u in range(NU):
                        for h in range(NH):
                            nc.vector.tensor_add(
                                otiles[u][:, h * N_HALF : (h + 1) * N_HALF],
                                pss[u * NH + h][:],
                                btile[:, h * N_HALF : (h + 1) * N_HALF],
                            )
                        store_eng = nc.scalar if u % 2 == 0 else nc.sync
                        store_eng.dma_start(out_v[t, u], otiles[u][:])
                    continue
                for u in range(NU):
                    otile = opool.tile([128, D_OUT], f32)
                    pss = []
                    for _h in range(NH):
                        ps = ppool.tile([128, N_HALF], f32, tag="ps")
                        pss.append(ps)
                    for k in range(KC):
                        # both output halves per k: consecutive matmuls
                        # share the stationary lhsT, halving LDW pressure
                        for h in range(NH):
                            nc.tensor.matmul(
                                pss[h][:],
                                xtile[:, k, u * 128 : (u + 1) * 128],  # lhsT [K,M]
                                wtile[:, k, h * N_HALF : (h + 1) * N_HALF],  # rhs [K,N]
                                start=(k == 0),
                                stop=(k == KC - 1),
                            )
                    for h in range(NH):
                        # evacuate psum with fused bias add
                        nc.vector.tensor_add(
                            otile[:, h * N_HALF : (h + 1) * N_HALF],
                            pss[h][:],
                            btile[:, h * N_HALF : (h + 1) * N_HALF],
                        )
                    if t == S // S_MACRO - 1:
                        # final macro: store halves on both rings as soon
                        # as each bias-add lands — halves the last flush
                        # the exit drain waits on
                        for h in range(NH):
                            eng = nc.scalar if h == 0 else nc.sync
                            eng.dma_start(
                                out_v[t, u][:, h * N_HALF : (h + 1) * N_HALF],
                                otile[:, h * N_HALF : (h + 1) * N_HALF],
                            )
                    else:
                        # alternate store rings to halve store-issue queuing
                        store_eng = nc.scalar if (t * 4 + u) % 2 == 0 else nc.sync
                        store_eng.dma_start(out_v[t, u], otile[:])

    nc.compile()
    return nc


def _get_program():
    global _PROGRAM
    if _PROGRAM is None:
        _PROGRAM = _build_program()
    return _PROGRAM


def kernel(x, task_ids, W, b, shared_A, shared_B, expert_A, expert_B, collab_w):
    global LAST_RESULTS
    x = np.asarray(x, dtype=np.float32)
    task_ids = np.asarray(task_ids)
    W = np.asarray(W, dtype=np.float32)
    b = np.asarray(b, dtype=np.float32)
    B = x.shape[0]
    assert B == N_CORES and x.shape[1:] == (S, D_IN)

    cw = np.float32(1.0 / (1.0 + np.exp(-np.float64(collab_w))))
    w_shared = (
        W
        + np.float32(cw * SCALING)
        * (np.asarray(shared_B, np.float32) @ np.asarray(shared_A, np.float32))
    ).astype(np.float32)
    ce = np.float32((1.0 - cw) * SCALING)

    np_in = mybir.dt.np(MM_DT)
    bb = np.ascontiguousarray(np.broadcast_to(b, (128, D_OUT)), dtype=np.float32)
    in_maps = []
    for bi in range(B):
        t = int(task_ids[bi])
        w_eff = w_shared + ce * (
            np.asarray(expert_B[t], np.float32) @ np.asarray(expert_A[t], np.float32)
        )
        in_maps.append(
            {
                "xt": np.ascontiguousarray(x[bi].T).astype(np_in),
                "wt": np.ascontiguousarray(w_eff.T).astype(np_in),
                "bb": bb,
            }
        )

    nc = _get_program()
    LAST_RESULTS = run_bass_kernel_spmd(nc, in_maps, list(range(N_CORES)))
    out = np.stack(
        [LAST_RESULTS.results[c]["out"] for c in range(N_CORES)], axis=0
    )
    return np.ascontiguousarray(out, dtype=np.float32)


# revision 21
# speedup vs baseline: 1.0271x; 1.0246x over previous
"""COLoRA linear kernel for 8 Trainium2 NeuronCores.

Reference computation (per batch element b with task t = task_ids[b]):

    out[b] = x[b] @ W.T + bias
           + cw      * 2 * (x[b] @ shared_A.T)    @ shared_B.T
           + (1-cw)  * 2 * (x[b] @ expert_A[t].T) @ expert_B[t].T
    cw = sigmoid(collab_w)

The rank-8 adapters fold exactly into the dense weight (associativity):

    W_eff[b] = W + cw*2*(shared_B @ shared_A) + (1-cw)*2*(expert_B[t] @ expert_A[t])
    out[b]   = x[b] @ W_eff[b].T + bias

so the device kernel is a single memory-bound GEMM per core. Sharding is
data-parallel over batch: core c handles batch element c (B == n_cores == 8).
The MoE routing (task_ids gather) happens on the host at dispatch time.

x is pre-transposed on the host to [d_in, s] so that the contraction dim
lands on SBUF partitions with fully contiguous DMA access patterns — no
on-chip transpose is needed.  Matmuls run in float32r (fp32 storage,
single-pass reduced-precision PE mode: 1 cycle/row at moving dim >= 256).
"""

import os

import numpy as np

import concourse.bass as bass
import concourse.tile as tile
from concourse import bacc, mybir
from concourse.bass_utils import run_bass_kernel_spmd

try:  # tracing (BASS_TRACE) needs the axon NTFF hook; scrub if unavailable
    from antenv.axon_hooks import get_axon_ntff_profile_hook  # noqa: F401
except ImportError:
    os.environ.pop("BASS_TRACE", None)

N_CORES = 8
S = 4096        # rows per core (sequence length; one batch element per core)
D_IN = 1024
D_OUT = 1024
KC = D_IN // 128   # contraction chunks of 128
S_MACRO = 512      # s rows loaded per x DMA
N_HALF = 512       # psum free dim (one bank)
SCALING = 2.0      # lora alpha/r = 16/8

# bf16 fallback switch (halves input DMA bytes, looser numerics). float32r
# measured accurate and memory-roofline-bound, so keep fp32 storage.
MM_DT = mybir.dt.float32r

_PROGRAM = None
LAST_RESULTS = None  # test harness introspection (exec_time_ns when traced)


def _build_program():
    f32 = mybir.dt.float32
    nc = bacc.Bacc("TRN2", debug=False, num_devices=N_CORES)

    xt_d = nc.dram_tensor("xt", [D_IN, S], MM_DT, kind="ExternalInput").ap()
    wt_d = nc.dram_tensor("wt", [D_IN, D_OUT], MM_DT, kind="ExternalInput").ap()
    bb_d = nc.dram_tensor("bb", [128, D_OUT], f32, kind="ExternalInput").ap()
    out_d = nc.dram_tensor("out", [S, D_OUT], f32, kind="ExternalOutput").ap()

    # contraction dim on partitions, chunked by 128
    xt_v = xt_d.rearrange("(k p) s -> p k s", p=128)      # [128, KC, S]
    wt_v = wt_d.rearrange("(k p) o -> p k o", p=128)      # [128, KC, D_OUT]
    # output rows s = t*S_MACRO + u*128 + p
    out_v = out_d.rearrange(
        "(t u p) o -> t u p o", u=S_MACRO // 128, p=128
    )  # [T, 4, 128, D_OUT]

    with tile.TileContext(nc) as tc:
        with (
            tc.tile_pool(name="const", bufs=1) as cpool,
            tc.tile_pool(name="xin", bufs=3) as xpool,
            tc.tile_pool(name="outp", bufs=4) as opool,
            tc.tile_pool(name="psum", bufs=8, space="PSUM") as ppool,
        ):
            # PE HAM warmup: dummy matmuls with no DMA deps ramp the PE
            # clock (1.2 -> 2.4 GHz takes ~3.4us of sustained activity)
            # while the first input DMAs are still in flight.
            warm_w = cpool.tile([128, 128], f32)
            warm_x = cpool.tile([128, 256], f32)
            nc.gpsimd.memset(warm_w[:], 0.0)
            nc.gpsimd.memset(warm_x[:], 0.0)
            warm_ps = ppool.tile([128, N_HALF], f32, tag="ps")
            for _ in range(12):
                nc.tensor.matmul(
                    warm_ps[:, :256], warm_w[:], warm_x[:], start=True, stop=True
                )

            # weights per k-chunk on the ACT HWDGE ring so chunk 0 is
            # available ~2us after issue instead of after the full 4MiB
            wtile = cpool.tile([128, KC, D_OUT], MM_DT)
            for k in range(KC):
                nc.scalar.dma_start(wtile[:, k, :], wt_v[:, k, :])
            btile = cpool.tile([128, D_OUT], f32)
            nc.scalar.dma_start(btile[:], bb_d[:])

            NU = S_MACRO // 128
            NH = D_OUT // N_HALF
            for t in range(S // S_MACRO):
                xtile = xpool.tile([128, KC, S_MACRO], MM_DT)
                # split loads: matmuls on early k chunks start before the
                # later chunks arrive (finest split on the first tile,
                # which gates the pipeline ramp)
                s_sl = slice(t * S_MACRO, (t + 1) * S_MACRO)
                if t == 0:
                    for k in range(KC):
                        nc.sync.dma_start(xtile[:, k, :], xt_v[:, k, s_sl])
                else:
                    nc.sync.dma_start(
                        xtile[:, : KC // 2, :], xt_v[:, : KC // 2, s_sl]
                    )
                    nc.sync.dma_start(
                        xtile[:, KC // 2 :, :], xt_v[:, KC // 2 :, s_sl]
                    )
                if t == 0:
                    # ramp macro: k outermost with all 8 psum groups open —
                    # each arriving (x[k], W[k]) chunk pair feeds 8 matmuls
                    # (~1.8us PE work per ~1.9us of DMA), so the PE never
                    # idles long enough to re-throttle while the front-load
                    # streams in.
                    otiles, pss = [], []
                    for u in range(NU):
                        otile = opool.tile([128, D_OUT], f32)
                        otiles.append(otile)
                        for _h in range(NH):
                            ps = ppool.tile([128, N_HALF], f32, tag="ps")
                            pss.append(ps)
                    for k in range(KC):
                        for u in range(NU):
                            for h in range(NH):
                                nc.tensor.matmul(
                                    pss[u * NH + h][:],
                                    xtile[:, k, u * 128 : (u + 1) * 128],
                                    wtile[:, k, h * N_HALF : (h + 1) * N_HALF],
                                    start=(k == 0),
                                    stop=(k == KC - 1),
                                )
                    for u in range(NU):
                        for h in range(NH):
                            nc.vector.tensor_add(
                                otiles[u][:, h * N_HALF : (h + 1) * N_HALF],
                                pss[u * NH + h][:],
                                btile[:, h * N_HALF : (h + 1) * N_HALF],
                            )
                        store_eng = nc.scalar if u % 2 == 0 else nc.sync
                        store_eng.dma_start(out_v[t, u], otiles[u][:])
                    continue
                for u in range(NU):
                    otile = opool.tile([128, D_OUT], f32)
                    pss = []
                    for _h in range(NH):
                        ps = ppool.tile([128, N_HALF], f32, tag="ps")
                        pss.append(ps)
                    for k in range(KC):
                        # both output halves per k: consecutive matmuls
                        # share the stationary lhsT, halving LDW pressure
                        for h in range(NH):
                            nc.tensor.matmul(
                                pss[h][:],
                                xtile[:, k, u * 128 : (u + 1) * 128],  # lhsT [K,M]
                                wtile[:, k, h * N_HALF : (h + 1) * N_HALF],  # rhs [K,N]
                                start=(k == 0),
                                stop=(k == KC - 1),
                            )
                    for h in range(NH):
                        # evacuate psum with fused bias add
                        nc.vector.tensor_add(
                            otile[:, h * N_HALF : (h + 1) * N_HALF],
                            pss[h][:],
                            btile[:, h * N_HALF : (h + 1) * N_HALF],
                        )
                    if t == S // S_MACRO - 1:
                        # final macro: store halves on both rings as soon
                        # as each bias-add lands — halves the last flush
                        # the exit drain waits on
                        for h in range(NH):
                            eng = nc.scalar if h == 0 else nc.sync
                            eng.dma_start(
                                out_v[t, u][:, h * N_HALF : (h + 1) * N_HALF],
                                otile[:, h * N_HALF : (h + 1) * N_HALF],
                            )
                    else:
                        # alternate store rings to halve store-issue queuing
                        store_eng = nc.scalar if (t * 4 + u) % 2 == 0 else nc.sync
                        store_eng.dma_start(out_v[t, u], otile[:])

    nc.compile()
    return nc


def _get_program():
    global _PROGRAM
    if _PROGRAM is None:
        _PROGRAM = _build_program()
    return _PROGRAM


def kernel(x, task_ids, W, b, shared_A, shared_B, expert_A, expert_B, collab_w):
    global LAST_RESULTS
    x = np.asarray(x, dtype=np.float32)
    task_ids = np.asarray(task_ids)
    W = np.asarray(W, dtype=np.float32)
    b = np.asarray(b, dtype=np.float32)
    B = x.shape[0]
    assert B == N_CORES and x.shape[1:] == (S, D_IN)

    cw = np.float32(1.0 / (1.0 + np.exp(-np.float64(collab_w))))
    w_shared = (
        W
        + np.float32(cw * SCALING)
        * (np.asarray(shared_B, np.float32) @ np.asarray(shared_A, np.float32))
    ).astype(np.float32)
    ce = np.float32((1.0 - cw) * SCALING)

    np_in = mybir.dt.np(MM_DT)
    bb = np.ascontiguousarray(np.broadcast_to(b, (128, D_OUT)), dtype=np.float32)
    in_maps = []
    for bi in range(B):
        t = int(task_ids[bi])
        w_eff = w_shared + ce * (
            np.asarray(expert_B[t], np.float32) @ np.asarray(expert_A[t], np.float32)
        )
        in_maps.append(
            {
                "xt": np.ascontiguousarray(x[bi].T).astype(np_in),
                "wt": np.ascontiguousarray(w_eff.T).astype(np_in),
                "bb": bb,
            }
        )

    nc = _get_program()
    LAST_RESULTS = run_bass_kernel_spmd(nc, in_maps, list(range(N_CORES)))
    out = np.stack(
        [LAST_RESULTS.results[c]["out"] for c in range(N_CORES)], axis=0
    )
    return np.ascontiguousarray(out, dtype=np.float32)
